# revision 21
# baseline (speedup 1.0000x reference)
"""Trainium2 Bass kernel for the LIF/hh neuron module.

Math (from the reference):
  fc = x @ W_fc.T + b_fc                    [B, T, C]
  per step t (state mem[B,C,4], spike[B,C]):
    x4   = mem[...,:3] @ w + b              (old mem)
    keep = DECAY * (1 - spike)
    mem03' = mem[...,:3]*keep + fc_t        (channels 0..2 identical updates!)
    mem3'  = mem[...,3]*keep + x4
    mem1 = mem03' @ w + b + mem3'
    spike' = mem1 > THRESH

Key identity: channels 0..2 of mem start at 0 and receive identical updates,
so m0==m1==m2 =: m for all t.  Let W = w0+w1+w2, u := W*m + b.  Then with
v_t := W * fc_t (folded into the GEMM weights on host), and b==0:
    w_t   = u + v_t                       (off critical chain)
    mem1' = DECAY*(mem1*n) + w_t          (chain; n := 1-spike)
    u'    = DECAY*(u*n) + v_t             (off chain)
    n'    = (mem1' <= THRESH)
State: (u, mem1, n).  Verified bit-identical to the reference recurrence.

GEMM in fp32r (fp22): 1 cycle/row for moving dim >= 256; ~2^-13 relative
error -> rel ~1e-2 on spikes (gate 2e-2).

Sharding (v2): 2-way tensor-parallel over out channels x 4-way data-parallel
over batch.  Core c: channel half h=c//4 (2048 ch), batch quarter q=c%4
(64 samples).  Per-core HBM traffic: W 32MB + x 15MB + out 7.5MB = 55MB
(~180us) vs the 205us tensor floor -> tensor-bound (the v1 bs=8 layout read
75MB/core and was DMA-bound).

Per-core GEMM: M=2048 (16 M-tiles), K=4096 (32 subtiles), N=960 (col=t*64+b),
psum split 512+448 (t-aligned 8t/7t; a single 960-wide matmul fails the ISA
s3d3_mm_num_elements check, so two per (j,s) it is).  x (15MB) resident,
streamed in 16 K-pieces on the sync queue; W streamed as 32 half-tiles
[128,16,128] (1MB) through a 4-slot ring on the scalar queue, with w4's
halves in dedicated boot slots loaded up front so phase 2 starts the moment
the last x piece is consumed.  Phase 1 (while x lands): matmuls emitted
piece-major across j0..j3 so the in-order PE queue consumes x pieces in
arrival order (~42us of work under the ~55us x load).  Phase 2: j-major,
prefetch w_{j+2} at iteration top, j4/j5 run all-A-then-all-B so psum-slot
WARs on the phase-1 copy train can't stall the PE.

Recurrence groups (4,4,4,2,2) M-tiles; each group's 15-step chain overlaps
the next group's GEMM.  Engine split is phase-dependent (measured, not
guessed): early groups (g0,g1) keep off-chain ops (w, uz) on GpSimd because
heavy DVE activity measurably slows the concurrent matmul stream (~18% in
an all-Vector experiment); late groups (g2, g3, tail) run all-Vector
because GpSimd pacing (~850ns/op + cross-engine sems, ~2.7us/step) would
serialize ahead of the tail on the in-order vector queue.  The tail group
(j14,j15, 128-wide, state shared with the big set via slices — safe since
all late recurrence ops are on the one in-order vector queue) is GEMM'd
A-halves-first/B-second so its t0..7 recurrence runs under the final GEMM
blocks and only the 7-step t8..14 chain trails the last matmul.

Measured 282.5us on HW (vs 439us baseline): ~25us preamble+ramp, ~239us
matmul span (205us fp32r floor + ~36us power throttle + 9us gaps), ~22us
recurrence tail.
"""
import sys
import os

sys.path.insert(0, "/opt/trn_rl_repo")

import numpy as np
import ml_dtypes

THRESH = 0.8
DECAY = 0.2

B, T, IN, C = 256, 15, 4096, 4096
NCORES = 8
CS = 2                    # channel shards
BS = 4                    # batch shards
MC = C // CS              # 2048 channels per core
BL = B // BS              # 64 samples per core
N = BL * T                # 960 moving columns (col = t*64 + b)
KS = IN // 128            # 32 K-subtiles
JC = MC // 128            # 16 M-tiles per core
PIECES = [(2 * i, 2) for i in range(16)]  # x K-pieces (s0, nsubtiles)
NA, NB = 512, 448         # psum split: t 0..7 | t 8..14
GROUPS = (4, 4, 4, 2, 2)  # recurrence group sizes in M-tiles
# last group (j14,j15) is GEMM'd A-first/B-second so its t0..7 recurrence
# overlaps the final GEMM blocks; only the 7-step B-half trails the GEMM

LAST_EXEC_S = None
LAST_NC = None            # stashed Bass module for test harness profiling


def _numpy_fallback(x, W_fc, b_fc, W_lif, b_lif):
    fc = np.einsum("bti,ci->btc", x.astype(np.float64), W_fc.astype(np.float64))
    fc += b_fc.astype(np.float64)
    w = W_lif[0].astype(np.float64)
    b = float(b_lif[0])
    Bs, Ts, Cs = fc.shape
    mem = np.zeros((Bs, Cs, 4))
    spike = np.zeros((Bs, Cs))
    outs = []
    for t in range(Ts):
        x4 = mem[..., :3] @ w + b
        keep = DECAY * (1.0 - spike)
        mem03 = mem[..., :3] * keep[..., None] + fc[:, t][..., None]
        mem3 = mem[..., 3] * keep + x4
        mem = np.concatenate([mem03, mem3[..., None]], axis=-1)
        mem1 = mem03 @ w + b + mem3
        spike = (mem1 > THRESH).astype(np.float64)
        outs.append(spike)
    return np.stack(outs, axis=1).astype(x.dtype)


def _legalize_waits(nc, mybir):
    """Walrus codegen caps embedded sync-waits per instruction (Matmult: 1,
    DMACopy: 2, ...).  Tile's sem assignment can exceed that.  Engines and
    DMA sequencers execute their queues in order, so moving excess waits onto
    freshly inserted same-engine NoOps directly before the instruction is
    semantically identical.  One wait per NoOp (NoOp capacity unknown)."""
    limits = {}
    counter = [0]
    for fn in nc.m.functions:
        for blk in fn.blocks:
            insts = blk.instructions
            out = []
            changed = False
            for inst in insts:
                tname = type(inst).__name__
                lim = limits.get(tname, 1)
                si = inst.sync_info
                waits = list(si.on_wait) if si is not None else []
                if len(waits) > lim:
                    excess, kept = waits[:-lim], waits[-lim:]
                    for w in excess:
                        counter[0] += 1
                        out.append(mybir.InstNoOp(
                            name=f"WSPLIT-{counter[0]}",
                            engine=inst.engine,
                            ins=[], outs=[],
                            sync_info=mybir.SyncInfo(on_wait=[w], on_update=[]),
                        ))
                    inst.sync_info = mybir.SyncInfo(
                        on_wait=kept, on_update=list(si.on_update))
                    changed = True
                out.append(inst)
            if changed:
                blk.instructions = out
    return counter[0]


def _build_bass():
    import concourse.bass as bass
    import concourse.mybir as mybir
    import concourse.tile as tile
    from contextlib import ExitStack

    f32 = mybir.dt.float32
    f32r = mybir.dt.float32r
    Alu = mybir.AluOpType

    nc = bass.Bass()
    wt_d = nc.dram_tensor("wt", [JC, 2, 128, 16, 128], f32r, kind="ExternalInput")
    xt_d = nc.dram_tensor("xt", [128, KS, N], f32r, kind="ExternalInput")
    sp_d = nc.dram_tensor("sp", [128, T, JC * BL], f32, kind="ExternalOutput")

    # group bookkeeping: j -> group index at start/end
    gstart = {}
    gend = {}
    j0 = 0
    for g, sz in enumerate(GROUPS):
        gstart[j0] = g
        gend[j0 + sz - 1] = g
        j0 += sz
    goffs = np.cumsum([0] + list(GROUPS))[:-1] * BL

    with ExitStack() as ctx:
        tc = ctx.enter_context(tile.TileContext(nc))
        xpool = ctx.enter_context(tc.tile_pool(name="xpool", bufs=1))
        wpool = ctx.enter_context(tc.tile_pool(name="wpool", bufs=4))
        wboot = ctx.enter_context(tc.tile_pool(name="wboot", bufs=1))
        fcpool = ctx.enter_context(tc.tile_pool(name="fcpool", bufs=2))
        spool = ctx.enter_context(tc.tile_pool(name="state", bufs=1))
        ppool = ctx.enter_context(tc.tile_pool(name="psum", bufs=4, space="PSUM"))

        # x resident, streamed in K-pieces on the sync queue; s -> piece slice
        xp = []
        s2piece = {}
        for i, (s0, ns) in enumerate(PIECES):
            t_ = xpool.tile([128, ns, N], f32r, tag=f"x{i}", name=f"x{i}")
            nc.sync.dma_start(t_[:], xt_d[:, s0:s0 + ns, :])
            xp.append(t_)
            for k in range(ns):
                s2piece[s0 + k] = (i, k)

        def rhs_of(s):
            i, k = s2piece[s]
            return xp[i][:, k, :]

        # weight half-tiles through a 4-slot ring on the scalar queue;
        # w4's halves get dedicated boot slots loaded up front so phase 2
        # can start the instant the last x piece is consumed
        wh = {}

        def load_wh(j, hf, pool=None, tag="wh"):
            pool = pool or wpool
            t_ = pool.tile([128, 16, 128], f32r, tag=tag, name=f"w{j}h{hf}")
            nc.scalar.dma_start(t_[:], wt_d[j, hf])
            wh[(j, hf)] = t_

        for j in range(4):
            load_wh(j, 0)
        load_wh(4, 0, pool=wboot, tag="wb0")
        load_wh(4, 1, pool=wboot, tag="wb1")
        for j in range(4):
            load_wh(j, 1)

        # shared state tiles; the tail group slices [:, :128] of the same
        # set — safe because every recurrence op runs on the in-order
        # vector queue, so cross-group WAR order is program order
        def state_set(pref, width):
            names = ("m1", "z", "uz", "w", "u0", "u1", "ns0", "ns1")
            return {nm: spool.tile([128, width], f32, tag=f"{pref}{nm}",
                                   name=f"{pref}{nm}") for nm in names}

        big_st = state_set("b_", 4 * BL)
        tail_st = {4: big_st}

        def emit_rec(g, fc_g, goff, gw, st, tail, t_lo, t_hi):
            m1 = st["m1"][:, :gw]
            z = st["z"][:, :gw]
            uz = st["uz"][:, :gw]
            w = st["w"][:, :gw]
            u_ = (st["u0"][:, :gw], st["u1"][:, :gw])
            ns_ = (st["ns0"][:, :gw], st["ns1"][:, :gw])
            if t_lo == 0:
                v0 = fc_g[:, 0, :]
                nc.vector.tensor_scalar_add(m1, v0, 0.0)
                nc.vector.tensor_scalar_add(u_[0], v0, 0.0)
                nc.vector.tensor_scalar(ns_[0], m1, THRESH, None, Alu.is_le)
                nc.sync.dma_start(sp_d[:, 0, goff:goff + gw], ns_[0])
                t_lo = 1
            for t in range(t_lo, t_hi):
                p, pp = t % 2, (t - 1) % 2
                vt = fc_g[:, t, :]
                last = (t == T - 1)  # u'/uz feed t+1 only; skip at the end
                if tail:
                    # late groups all-Vector: gpsimd pacing (~850ns/op +
                    # cross-engine sems) would serialize ahead of the tail
                    nc.vector.tensor_tensor(w, u_[pp], vt, Alu.add)
                    nc.vector.tensor_tensor(z, m1, ns_[pp], Alu.mult)
                    nc.vector.scalar_tensor_tensor(
                        m1, z, DECAY, w, Alu.mult, Alu.add)
                    nc.vector.tensor_scalar(ns_[p], m1, THRESH, None, Alu.is_le)
                    if not last:
                        nc.vector.tensor_tensor(uz, u_[pp], ns_[pp], Alu.mult)
                        nc.vector.scalar_tensor_tensor(
                            u_[p], uz, DECAY, vt, Alu.mult, Alu.add)
                else:
                    # early groups split across engines: heavy DVE activity
                    # measurably slows the concurrent matmul stream, so
                    # keep vector occupancy low while the GEMM runs
                    nc.gpsimd.tensor_tensor(w, u_[pp], vt, Alu.add)
                    if not last:
                        nc.gpsimd.tensor_tensor(uz, u_[pp], ns_[pp], Alu.mult)
                    nc.vector.tensor_tensor(z, m1, ns_[pp], Alu.mult)
                    nc.vector.scalar_tensor_tensor(
                        m1, z, DECAY, w, Alu.mult, Alu.add)
                    nc.vector.tensor_scalar(ns_[p], m1, THRESH, None, Alu.is_le)
                    if not last:
                        nc.vector.scalar_tensor_tensor(
                            u_[p], uz, DECAY, vt, Alu.mult, Alu.add)
                nc.sync.dma_start(sp_d[:, t, goff:goff + gw], ns_[p])

        # phase-1 matmuls: piece-major over j0..j3 so the PE stream consumes
        # x pieces in arrival order (PE executes its queue in order)
        ps = {}
        for j in range(4):
            ps[j] = (ppool.tile([128, NA], f32, tag="psA", name=f"psA{j}",
                                bufs=5),
                     ppool.tile([128, NB], f32, tag="psB", name=f"psB{j}",
                                bufs=3))
        for i, (s0, nsub) in enumerate(PIECES):
            for j in range(4):
                for k in range(nsub):
                    s = s0 + k
                    lhsT = wh[(j, s // 16)][:, s % 16, :]
                    rhs = xp[i][:, k, :]
                    nc.tensor.matmul(ps[j][0], lhsT, rhs[:, 0:NA],
                                     start=(s == 0), stop=(s == KS - 1))
                    nc.tensor.matmul(ps[j][1], lhsT, rhs[:, NA:N],
                                     start=(s == 0), stop=(s == KS - 1))

        def alloc_ps(j):
            psA = ppool.tile([128, NA], f32, tag="psA", name=f"psA{j}", bufs=5)
            psB = ppool.tile([128, NB], f32, tag="psB", name=f"psB{j}", bufs=3)
            ps[j] = (psA, psB)
            return psA, psB

        def mm_half(j, psX, lo, hi, srange):
            for s in srange:
                nc.tensor.matmul(psX, wh[(j, s // 16)][:, s % 16, :],
                                 rhs_of(s)[:, lo:hi],
                                 start=(s == 0), stop=(s == KS - 1))

        def copy_half(j, half):
            g = [g_ for jj_, g_ in gstart.items() if jj_ <= j][-1]
            jj = j * BL - int(goffs[g])
            psX = ps[j][half]
            t0, t1 = (0, 8) if half == 0 else (8, T)
            nc.scalar.copy(fc_of[g][:, t0:t1, jj:jj + BL],
                           psX.rearrange("p (t b) -> p t b", b=BL))

        fc_of = {}
        for j in range(JC - 2):
            if j in gstart:
                g = gstart[j]
                fc_of[g] = fcpool.tile([128, T, GROUPS[g] * BL], f32,
                                       tag="fc", name=f"fc{g}")
            if j >= 4:
                # prefetch at iteration top: by the time the Act sequencer
                # reaches this trigger its slot-WAR is satisfied, and the
                # transfer lands an iteration before j+2 needs it
                if j <= 13:
                    load_wh(j + 2, 0)
                    load_wh(j + 2, 1)
                psA, psB = alloc_ps(j)
                if j <= 5:
                    # A-run fully before B-run: j4/j5's psB slot WARs a
                    # phase-1 copy that lands only at x-done; the 6.8us
                    # A-run hides that wait instead of stalling the PE
                    for s in range(KS):
                        nc.tensor.matmul(psA, wh[(j, s // 16)][:, s % 16, :],
                                         rhs_of(s)[:, 0:NA],
                                         start=(s == 0), stop=(s == KS - 1))
                    for s in range(KS):
                        nc.tensor.matmul(psB, wh[(j, s // 16)][:, s % 16, :],
                                         rhs_of(s)[:, NA:N],
                                         start=(s == 0), stop=(s == KS - 1))
                else:
                    for s in range(KS):
                        lhsT = wh[(j, s // 16)][:, s % 16, :]
                        rhs = rhs_of(s)
                        nc.tensor.matmul(psA, lhsT, rhs[:, 0:NA],
                                         start=(s == 0), stop=(s == KS - 1))
                        nc.tensor.matmul(psB, lhsT, rhs[:, NA:N],
                                         start=(s == 0), stop=(s == KS - 1))
            copy_half(j, 0)
            copy_half(j, 1)
            if j == 3:
                load_wh(5, 0)
                load_wh(5, 1)
            if j in gend:
                g = gend[j]
                emit_rec(g, fc_of[g], int(goffs[g]), GROUPS[g] * BL,
                         big_st, g >= 2, 0, T)

        # tail group (j14, j15): A-halves first (s interleaved with the weight
        # ring: h0 blocks then h1 blocks), then B-halves; the t0..7 recurrence
        # runs under the B GEMM blocks, so only t8..14 trails the last matmul
        g = gstart[JC - 2]
        fc_of[g] = fcpool.tile([128, T, 2 * BL], f32, tag="fc", name=f"fc{g}")
        pA14, pB14 = alloc_ps(JC - 2)
        pA15, pB15 = alloc_ps(JC - 1)
        mm_half(JC - 2, pA14, 0, NA, range(0, 16))
        mm_half(JC - 1, pA15, 0, NA, range(0, 16))
        mm_half(JC - 2, pA14, 0, NA, range(16, KS))
        mm_half(JC - 1, pA15, 0, NA, range(16, KS))
        copy_half(JC - 2, 0)
        copy_half(JC - 1, 0)
        emit_rec(g, fc_of[g], int(goffs[g]), 2 * BL, tail_st[4], True, 0, 8)
        mm_half(JC - 2, pB14, NA, N, range(0, 16))
        mm_half(JC - 1, pB15, NA, N, range(0, 16))
        mm_half(JC - 2, pB14, NA, N, range(16, KS))
        mm_half(JC - 1, pB15, NA, N, range(16, KS))
        copy_half(JC - 2, 1)
        copy_half(JC - 1, 1)
        emit_rec(g, fc_of[g], int(goffs[g]), 2 * BL, tail_st[4], True, 8, T)
    _legalize_waits(nc, mybir)
    return nc


_CACHE = {}


def _get_runner():
    """Compile once; return (fn, in_names, out_names, zero_outs, mesh)."""
    if "fn" in _CACHE:
        return _CACHE["fn"]
    global LAST_NC
    import jax
    import numpy as _np
    from jax.sharding import Mesh, PartitionSpec
    from jax.experimental.shard_map import shard_map
    import concourse.mybir as mybir
    from concourse import bass2jax

    bass2jax.install_neuronx_cc_hook()
    nc = _build_bass()
    LAST_NC = nc

    in_names, out_names, out_avals, zero_outs = [], [], [], []
    partition_name = nc.partition_id_tensor.name if nc.partition_id_tensor else None
    for alloc in nc.m.functions[0].allocations:
        if not isinstance(alloc, mybir.MemoryLocationSet):
            continue
        name = alloc.memorylocations[0].name
        if alloc.kind == "ExternalInput":
            if name != partition_name:
                in_names.append(name)
        elif alloc.kind == "ExternalOutput":
            shape = tuple(alloc.tensor_shape)
            dtype = mybir.dt.np(alloc.dtype)
            out_names.append(name)
            out_avals.append(jax.core.ShapedArray(shape, dtype))
            zero_outs.append(_np.zeros(shape, dtype))
    n_params = len(in_names)
    all_in_names = list(in_names) + list(out_names)
    if partition_name is not None:
        all_in_names.append(partition_name)
    donate = tuple(range(n_params, n_params + len(out_names)))

    def _body(*args):
        operands = list(args)
        if partition_name is not None:
            operands.append(bass2jax.partition_id_tensor())
        outs = bass2jax._bass_exec_p.bind(
            *operands,
            out_avals=tuple(out_avals),
            in_names=tuple(all_in_names),
            out_names=tuple(out_names),
            lowering_input_output_aliases=(),
            sim_require_finite=True,
            sim_require_nnan=True,
            nc=nc,
        )
        return tuple(outs)

    devices = jax.devices()[:NCORES]
    mesh = Mesh(_np.asarray(devices), ("core",))
    n_all = n_params + len(out_names)
    sharded = jax.jit(
        shard_map(_body, mesh=mesh,
                  in_specs=(PartitionSpec("core"),) * n_all,
                  out_specs=(PartitionSpec("core"),) * len(out_names),
                  check_rep=False),
        donate_argnums=donate, keep_unused=True,
    )
    _CACHE["fn"] = (sharded, in_names, out_names, zero_outs, mesh)
    return _CACHE["fn"]


def kernel(x, W_fc, b_fc, W_lif, b_lif):
    global LAST_EXEC_S
    if np.any(b_fc != 0) or np.any(b_lif != 0):
        return _numpy_fallback(x, W_fc, b_fc, W_lif, b_lif)
    import time
    import jax

    Ws = float(W_lif[0, 0]) + float(W_lif[0, 1]) + float(W_lif[0, 2])
    # lhsT layout: wt[j, hf, p, sh, m] = (Ws*W_fc).T[(hf*16+sh)*128+p,
    #                                               h*2048 + j*128 + m]
    Wt = np.ascontiguousarray((W_fc.astype(np.float32) * np.float32(Ws)).T)
    wt_h = []
    for h in range(CS):
        arr = Wt[:, h * MC:(h + 1) * MC].reshape(2, 16, 128, JC, 128)
        wt_h.append(np.ascontiguousarray(arr.transpose(3, 0, 2, 1, 4)))

    per_core = {"wt": [], "xt": []}
    for c in range(NCORES):
        h, q = c // BS, c % BS
        xs = np.ascontiguousarray(
            x[q * BL:(q + 1) * BL].astype(np.float32).transpose(2, 1, 0)
        ).reshape(KS, 128, N)  # [s, p, t*BL+b]
        per_core["xt"].append(np.ascontiguousarray(xs.transpose(1, 0, 2)))
        per_core["wt"].append(wt_h[h])

    sharded, in_names, out_names, zero_outs, mesh = _get_runner()
    concat_in = [np.concatenate(per_core[n], axis=0) for n in in_names]
    concat_zero = [np.concatenate([z] * NCORES, axis=0) for z in zero_outs]

    from jax.sharding import NamedSharding, PartitionSpec
    shd = NamedSharding(mesh, PartitionSpec("core"))
    args = [jax.device_put(a, shd) for a in concat_in + concat_zero]
    for a in args:
        a.block_until_ready()
    t0 = time.time()
    out_arrs = sharded(*args)
    jax.block_until_ready(out_arrs)
    LAST_EXEC_S = time.time() - t0
    out_arrs = [np.asarray(o) for o in out_arrs]

    sp_all = out_arrs[out_names.index("sp")]            # [8*128, T, JC*BL]
    out = np.empty((B, T, C), dtype=np.float32)
    for c in range(NCORES):
        h, q = c // BS, c % BS
        sp = sp_all[c * 128:(c + 1) * 128]              # [128, T, JC*BL]
        arr = sp.reshape(128, T, JC, BL)                # (p, t, j, b)
        spikes = 1.0 - np.transpose(arr, (3, 1, 2, 0))  # (b, t, j, p)
        out[q * BL:(q + 1) * BL, :, h * MC:(h + 1) * MC] = \
            spikes.reshape(BL, T, MC)
    return out


# revision 23
# speedup vs baseline: 1.0289x; 1.0289x over previous
"""Trainium2 Bass kernel for the LIF/hh neuron module.

Math (from the reference):
  fc = x @ W_fc.T + b_fc                    [B, T, C]
  per step t (state mem[B,C,4], spike[B,C]):
    x4   = mem[...,:3] @ w + b              (old mem)
    keep = DECAY * (1 - spike)
    mem03' = mem[...,:3]*keep + fc_t        (channels 0..2 identical updates!)
    mem3'  = mem[...,3]*keep + x4
    mem1 = mem03' @ w + b + mem3'
    spike' = mem1 > THRESH

Key identity: channels 0..2 of mem start at 0 and receive identical updates,
so m0==m1==m2 =: m for all t.  Let W = w0+w1+w2, u := W*m + b.  Then with
v_t := W * fc_t (folded into the GEMM weights on host), and b==0:
    w_t   = u + v_t                       (off critical chain)
    mem1' = DECAY*(mem1*n) + w_t          (chain; n := 1-spike)
    u'    = DECAY*(u*n) + v_t             (off chain)
    n'    = (mem1' <= THRESH)
State: (u, mem1, n).  Verified bit-identical to the reference recurrence.

GEMM in fp32r (fp22): 1 cycle/row for moving dim >= 256; ~2^-13 relative
error -> rel ~1e-2 on spikes (gate 2e-2).

Sharding (v2): 2-way tensor-parallel over out channels x 4-way data-parallel
over batch.  Core c: channel half h=c//4 (2048 ch), batch quarter q=c%4
(64 samples).  Per-core HBM traffic: W 32MB + x 15MB + out 7.5MB = 55MB
(~180us) vs the 205us tensor floor -> tensor-bound (the v1 bs=8 layout read
75MB/core and was DMA-bound).

Per-core GEMM: M=2048 (16 M-tiles), K=4096 (32 subtiles), N=960 (col=t*64+b),
psum split 512+448 (t-aligned 8t/7t; a single 960-wide matmul fails the ISA
s3d3_mm_num_elements check, so two per (j,s) it is).  x (15MB) resident,
streamed in 16 K-pieces on the sync queue; W streamed as 32 half-tiles
[128,16,128] (1MB) through a 4-slot ring on the scalar queue, with w4's
halves in dedicated boot slots loaded up front so phase 2 starts the moment
the last x piece is consumed.  Phase 1 (while x lands): matmuls emitted
piece-major across j0..j3 so the in-order PE queue consumes x pieces in
arrival order (~42us of work under the ~55us x load).  Phase 2: j-major,
prefetch w_{j+2} at iteration top, j4/j5 run all-A-then-all-B so psum-slot
WARs on the phase-1 copy train can't stall the PE.

Recurrence groups (4,4,4,2,2) M-tiles; each group's 15-step chain overlaps
the next group's GEMM.  Engine split is phase-dependent (measured, not
guessed): early groups (g0,g1) keep off-chain ops (w, uz) on GpSimd because
heavy DVE activity measurably slows the concurrent matmul stream (~18% in
an all-Vector experiment); late groups (g2, g3, tail) run all-Vector
because GpSimd pacing (~850ns/op + cross-engine sems, ~2.7us/step) would
serialize ahead of the tail on the in-order vector queue.  The tail group
(j14,j15, 128-wide, state shared with the big set via slices — safe since
all late recurrence ops are on the one in-order vector queue) is GEMM'd
A-halves-first/B-second so its t0..7 recurrence runs under the final GEMM
blocks and only the 7-step t8..14 chain trails the last matmul.

Measured 282.5us on HW (vs 439us baseline): ~25us preamble+ramp, ~239us
matmul span (205us fp32r floor + ~36us power throttle + 9us gaps), ~22us
recurrence tail.
"""
import sys
import os

sys.path.insert(0, "/opt/trn_rl_repo")

import numpy as np
import ml_dtypes

THRESH = 0.8
DECAY = 0.2

B, T, IN, C = 256, 15, 4096, 4096
NCORES = 8
CS = 2                    # channel shards
BS = 4                    # batch shards
MC = C // CS              # 2048 channels per core
BL = B // BS              # 64 samples per core
N = BL * T                # 960 moving columns (col = t*64 + b)
KS = IN // 128            # 32 K-subtiles
JC = MC // 128            # 16 M-tiles per core
# x K-pieces (s0, nsubtiles): first piece bigger so the cold-clock PE
# doesn't drain it before piece 2 arrives
PIECES = [(0, 4)] + [(4 + 2 * i, 2) for i in range(14)]
NA, NB = 512, 448         # psum split: t 0..7 | t 8..14
GROUPS = (4, 4, 4, 2, 2)  # recurrence group sizes in M-tiles
# last group (j14,j15) is GEMM'd A-first/B-second so its t0..7 recurrence
# overlaps the final GEMM blocks; only the 7-step B-half trails the GEMM

LAST_EXEC_S = None
LAST_NC = None            # stashed Bass module for test harness profiling


def _numpy_fallback(x, W_fc, b_fc, W_lif, b_lif):
    fc = np.einsum("bti,ci->btc", x.astype(np.float64), W_fc.astype(np.float64))
    fc += b_fc.astype(np.float64)
    w = W_lif[0].astype(np.float64)
    b = float(b_lif[0])
    Bs, Ts, Cs = fc.shape
    mem = np.zeros((Bs, Cs, 4))
    spike = np.zeros((Bs, Cs))
    outs = []
    for t in range(Ts):
        x4 = mem[..., :3] @ w + b
        keep = DECAY * (1.0 - spike)
        mem03 = mem[..., :3] * keep[..., None] + fc[:, t][..., None]
        mem3 = mem[..., 3] * keep + x4
        mem = np.concatenate([mem03, mem3[..., None]], axis=-1)
        mem1 = mem03 @ w + b + mem3
        spike = (mem1 > THRESH).astype(np.float64)
        outs.append(spike)
    return np.stack(outs, axis=1).astype(x.dtype)


def _legalize_waits(nc, mybir):
    """Walrus codegen caps embedded sync-waits per instruction (Matmult: 1,
    DMACopy: 2, ...).  Tile's sem assignment can exceed that.  Engines and
    DMA sequencers execute their queues in order, so moving excess waits onto
    freshly inserted same-engine NoOps directly before the instruction is
    semantically identical.  One wait per NoOp (NoOp capacity unknown)."""
    limits = {}
    counter = [0]
    for fn in nc.m.functions:
        for blk in fn.blocks:
            insts = blk.instructions
            out = []
            changed = False
            for inst in insts:
                tname = type(inst).__name__
                lim = limits.get(tname, 1)
                si = inst.sync_info
                waits = list(si.on_wait) if si is not None else []
                if len(waits) > lim:
                    excess, kept = waits[:-lim], waits[-lim:]
                    for w in excess:
                        counter[0] += 1
                        out.append(mybir.InstNoOp(
                            name=f"WSPLIT-{counter[0]}",
                            engine=inst.engine,
                            ins=[], outs=[],
                            sync_info=mybir.SyncInfo(on_wait=[w], on_update=[]),
                        ))
                    inst.sync_info = mybir.SyncInfo(
                        on_wait=kept, on_update=list(si.on_update))
                    changed = True
                out.append(inst)
            if changed:
                blk.instructions = out
    return counter[0]


def _build_bass():
    import concourse.bass as bass
    import concourse.mybir as mybir
    import concourse.tile as tile
    from contextlib import ExitStack

    f32 = mybir.dt.float32
    f32r = mybir.dt.float32r
    Alu = mybir.AluOpType

    nc = bass.Bass()
    wt_d = nc.dram_tensor("wt", [JC, 2, 128, 16, 128], f32r, kind="ExternalInput")
    xt_d = nc.dram_tensor("xt", [128, KS, N], f32r, kind="ExternalInput")
    sp_d = nc.dram_tensor("sp", [128, T, JC * BL], f32, kind="ExternalOutput")

    # group bookkeeping: j -> group index at start/end
    gstart = {}
    gend = {}
    j0 = 0
    for g, sz in enumerate(GROUPS):
        gstart[j0] = g
        gend[j0 + sz - 1] = g
        j0 += sz
    goffs = np.cumsum([0] + list(GROUPS))[:-1] * BL

    with ExitStack() as ctx:
        tc = ctx.enter_context(tile.TileContext(nc))
        xpool = ctx.enter_context(tc.tile_pool(name="xpool", bufs=1))
        wpool = ctx.enter_context(tc.tile_pool(name="wpool", bufs=4))
        wboot = ctx.enter_context(tc.tile_pool(name="wboot", bufs=1))
        fcpool = ctx.enter_context(tc.tile_pool(name="fcpool", bufs=2))
        spool = ctx.enter_context(tc.tile_pool(name="state", bufs=1))
        ppool = ctx.enter_context(tc.tile_pool(name="psum", bufs=4, space="PSUM"))

        # x resident, streamed in K-pieces on the sync queue; s -> piece slice
        xp = []
        s2piece = {}
        for i, (s0, ns) in enumerate(PIECES):
            t_ = xpool.tile([128, ns, N], f32r, tag=f"x{i}", name=f"x{i}")
            nc.sync.dma_start(t_[:], xt_d[:, s0:s0 + ns, :])
            xp.append(t_)
            for k in range(ns):
                s2piece[s0 + k] = (i, k)

        def rhs_of(s):
            i, k = s2piece[s]
            return xp[i][:, k, :]

        # weight half-tiles through a 4-slot ring on the scalar queue;
        # w4's halves get dedicated boot slots loaded up front so phase 2
        # can start the instant the last x piece is consumed
        wh = {}

        def load_wh(j, hf, pool=None, tag="wh"):
            pool = pool or wpool
            t_ = pool.tile([128, 16, 128], f32r, tag=tag, name=f"w{j}h{hf}")
            nc.scalar.dma_start(t_[:], wt_d[j, hf])
            wh[(j, hf)] = t_

        for j in range(4):
            load_wh(j, 0)
        load_wh(4, 0, pool=wboot, tag="wb0")
        load_wh(4, 1, pool=wboot, tag="wb1")
        for j in range(4):
            load_wh(j, 1)

        # shared state tiles; the tail group slices [:, :128] of the same
        # set — safe because every recurrence op runs on the in-order
        # vector queue, so cross-group WAR order is program order
        def state_set(pref, width):
            names = ("m1", "z", "uz", "w", "u0", "u1", "ns0", "ns1")
            return {nm: spool.tile([128, width], f32, tag=f"{pref}{nm}",
                                   name=f"{pref}{nm}") for nm in names}

        big_st = state_set("b_", 4 * BL)
        tail_st = {4: big_st}

        def emit_rec(g, fc_g, goff, gw, st, tail, t_lo, t_hi):
            m1 = st["m1"][:, :gw]
            z = st["z"][:, :gw]
            uz = st["uz"][:, :gw]
            w = st["w"][:, :gw]
            u_ = (st["u0"][:, :gw], st["u1"][:, :gw])
            ns_ = (st["ns0"][:, :gw], st["ns1"][:, :gw])
            if t_lo == 0:
                v0 = fc_g[:, 0, :]
                nc.vector.tensor_scalar_add(m1, v0, 0.0)
                nc.vector.tensor_scalar_add(u_[0], v0, 0.0)
                nc.vector.tensor_scalar(ns_[0], m1, THRESH, None, Alu.is_le)
                nc.sync.dma_start(sp_d[:, 0, goff:goff + gw], ns_[0])
                t_lo = 1
            for t in range(t_lo, t_hi):
                p, pp = t % 2, (t - 1) % 2
                vt = fc_g[:, t, :]
                last = (t == T - 1)  # u'/uz feed t+1 only; skip at the end
                if tail:
                    # late groups all-Vector: gpsimd pacing (~850ns/op +
                    # cross-engine sems) would serialize ahead of the tail
                    nc.vector.tensor_tensor(w, u_[pp], vt, Alu.add)
                    nc.vector.tensor_tensor(z, m1, ns_[pp], Alu.mult)
                    nc.vector.scalar_tensor_tensor(
                        m1, z, DECAY, w, Alu.mult, Alu.add)
                    nc.vector.tensor_scalar(ns_[p], m1, THRESH, None, Alu.is_le)
                    if not last:
                        nc.vector.tensor_tensor(uz, u_[pp], ns_[pp], Alu.mult)
                        nc.vector.scalar_tensor_tensor(
                            u_[p], uz, DECAY, vt, Alu.mult, Alu.add)
                else:
                    # early groups split across engines: heavy DVE activity
                    # measurably slows the concurrent matmul stream, so
                    # keep vector occupancy low while the GEMM runs
                    nc.gpsimd.tensor_tensor(w, u_[pp], vt, Alu.add)
                    if not last:
                        nc.gpsimd.tensor_tensor(uz, u_[pp], ns_[pp], Alu.mult)
                    nc.vector.tensor_tensor(z, m1, ns_[pp], Alu.mult)
                    nc.vector.scalar_tensor_tensor(
                        m1, z, DECAY, w, Alu.mult, Alu.add)
                    nc.vector.tensor_scalar(ns_[p], m1, THRESH, None, Alu.is_le)
                    if not last:
                        nc.vector.scalar_tensor_tensor(
                            u_[p], uz, DECAY, vt, Alu.mult, Alu.add)
                nc.sync.dma_start(sp_d[:, t, goff:goff + gw], ns_[p])

        # phase-1 matmuls: piece-major over j0..j3 so the PE stream consumes
        # x pieces in arrival order (PE executes its queue in order)
        ps = {}
        for j in range(4):
            ps[j] = (ppool.tile([128, NA], f32, tag="psA", name=f"psA{j}",
                                bufs=5),
                     ppool.tile([128, NB], f32, tag="psB", name=f"psB{j}",
                                bufs=3))
        for i, (s0, nsub) in enumerate(PIECES):
            for j in range(4):
                for k in range(nsub):
                    s = s0 + k
                    lhsT = wh[(j, s // 16)][:, s % 16, :]
                    rhs = xp[i][:, k, :]
                    nc.tensor.matmul(ps[j][0], lhsT, rhs[:, 0:NA],
                                     start=(s == 0), stop=(s == KS - 1))
                    nc.tensor.matmul(ps[j][1], lhsT, rhs[:, NA:N],
                                     start=(s == 0), stop=(s == KS - 1))

        def alloc_ps(j):
            psA = ppool.tile([128, NA], f32, tag="psA", name=f"psA{j}", bufs=5)
            psB = ppool.tile([128, NB], f32, tag="psB", name=f"psB{j}", bufs=3)
            ps[j] = (psA, psB)
            return psA, psB

        def mm_half(j, psX, lo, hi, srange):
            for s in srange:
                nc.tensor.matmul(psX, wh[(j, s // 16)][:, s % 16, :],
                                 rhs_of(s)[:, lo:hi],
                                 start=(s == 0), stop=(s == KS - 1))

        def copy_half(j, half):
            g = [g_ for jj_, g_ in gstart.items() if jj_ <= j][-1]
            jj = j * BL - int(goffs[g])
            psX = ps[j][half]
            t0, t1 = (0, 8) if half == 0 else (8, T)
            nc.scalar.copy(fc_of[g][:, t0:t1, jj:jj + BL],
                           psX.rearrange("p (t b) -> p t b", b=BL))

        fc_of = {}
        for j in range(JC - 2):
            if j in gstart:
                g = gstart[j]
                fc_of[g] = fcpool.tile([128, T, GROUPS[g] * BL], f32,
                                       tag="fc", name=f"fc{g}")
            if j >= 4:
                # prefetch at iteration top: by the time the Act sequencer
                # reaches this trigger its slot-WAR is satisfied, and the
                # transfer lands an iteration before j+2 needs it
                if j <= 13:
                    load_wh(j + 2, 0)
                    load_wh(j + 2, 1)
                psA, psB = alloc_ps(j)
                # A-run fully before B-run, for every tile: (a) j4/j5's psB
                # slot WARs a phase-1 copy that lands only at x-done, and
                # the A-run hides that wait; (b) a group-ending tile's
                # A-copy lands ~6us before its B-copy, so the group's t0..7
                # recurrence can be emitted between the halves and start
                # that much earlier — this shifts the whole serialized
                # late-group recurrence pile forward.  Per-bank s-order is
                # unchanged, so results are bit-identical.
                for s in range(KS):
                    nc.tensor.matmul(psA, wh[(j, s // 16)][:, s % 16, :],
                                     rhs_of(s)[:, 0:NA],
                                     start=(s == 0), stop=(s == KS - 1))
                copy_half(j, 0)
                if j in gend:
                    emit_rec(gend[j], fc_of[gend[j]], int(goffs[gend[j]]),
                             GROUPS[gend[j]] * BL, big_st, gend[j] >= 2, 0, 8)
                for s in range(KS):
                    nc.tensor.matmul(psB, wh[(j, s // 16)][:, s % 16, :],
                                     rhs_of(s)[:, NA:N],
                                     start=(s == 0), stop=(s == KS - 1))
                copy_half(j, 1)
                if j in gend:
                    emit_rec(gend[j], fc_of[gend[j]], int(goffs[gend[j]]),
                             GROUPS[gend[j]] * BL, big_st, gend[j] >= 2, 8, T)
            else:
                copy_half(j, 0)
                copy_half(j, 1)
                if j in gend:
                    g = gend[j]
                    emit_rec(g, fc_of[g], int(goffs[g]), GROUPS[g] * BL,
                             big_st, g >= 2, 0, T)
            if j == 3:
                load_wh(5, 0)
                load_wh(5, 1)

        # tail group (j14, j15): A-halves first (s interleaved with the weight
        # ring: h0 blocks then h1 blocks), then B-halves; the t0..7 recurrence
        # runs under the B GEMM blocks, so only t8..14 trails the last matmul
        g = gstart[JC - 2]
        fc_of[g] = fcpool.tile([128, T, 2 * BL], f32, tag="fc", name=f"fc{g}")
        pA14, pB14 = alloc_ps(JC - 2)
        pA15, pB15 = alloc_ps(JC - 1)
        mm_half(JC - 2, pA14, 0, NA, range(0, 16))
        mm_half(JC - 1, pA15, 0, NA, range(0, 16))
        mm_half(JC - 2, pA14, 0, NA, range(16, KS))
        mm_half(JC - 1, pA15, 0, NA, range(16, KS))
        copy_half(JC - 2, 0)
        copy_half(JC - 1, 0)
        emit_rec(g, fc_of[g], int(goffs[g]), 2 * BL, tail_st[4], True, 0, 8)
        mm_half(JC - 2, pB14, NA, N, range(0, 16))
        mm_half(JC - 1, pB15, NA, N, range(0, 16))
        mm_half(JC - 2, pB14, NA, N, range(16, KS))
        mm_half(JC - 1, pB15, NA, N, range(16, KS))
        copy_half(JC - 2, 1)
        copy_half(JC - 1, 1)
        emit_rec(g, fc_of[g], int(goffs[g]), 2 * BL, tail_st[4], True, 8, T)
    _legalize_waits(nc, mybir)
    return nc


_CACHE = {}


def _get_runner():
    """Compile once; return (fn, in_names, out_names, zero_outs, mesh)."""
    if "fn" in _CACHE:
        return _CACHE["fn"]
    global LAST_NC
    import jax
    import numpy as _np
    from jax.sharding import Mesh, PartitionSpec
    from jax.experimental.shard_map import shard_map
    import concourse.mybir as mybir
    from concourse import bass2jax

    bass2jax.install_neuronx_cc_hook()
    nc = _build_bass()
    LAST_NC = nc

    in_names, out_names, out_avals, zero_outs = [], [], [], []
    partition_name = nc.partition_id_tensor.name if nc.partition_id_tensor else None
    for alloc in nc.m.functions[0].allocations:
        if not isinstance(alloc, mybir.MemoryLocationSet):
            continue
        name = alloc.memorylocations[0].name
        if alloc.kind == "ExternalInput":
            if name != partition_name:
                in_names.append(name)
        elif alloc.kind == "ExternalOutput":
            shape = tuple(alloc.tensor_shape)
            dtype = mybir.dt.np(alloc.dtype)
            out_names.append(name)
            out_avals.append(jax.core.ShapedArray(shape, dtype))
            zero_outs.append(_np.zeros(shape, dtype))
    n_params = len(in_names)
    all_in_names = list(in_names) + list(out_names)
    if partition_name is not None:
        all_in_names.append(partition_name)
    donate = tuple(range(n_params, n_params + len(out_names)))

    def _body(*args):
        operands = list(args)
        if partition_name is not None:
            operands.append(bass2jax.partition_id_tensor())
        outs = bass2jax._bass_exec_p.bind(
            *operands,
            out_avals=tuple(out_avals),
            in_names=tuple(all_in_names),
            out_names=tuple(out_names),
            lowering_input_output_aliases=(),
            sim_require_finite=True,
            sim_require_nnan=True,
            nc=nc,
        )
        return tuple(outs)

    devices = jax.devices()[:NCORES]
    mesh = Mesh(_np.asarray(devices), ("core",))
    n_all = n_params + len(out_names)
    sharded = jax.jit(
        shard_map(_body, mesh=mesh,
                  in_specs=(PartitionSpec("core"),) * n_all,
                  out_specs=(PartitionSpec("core"),) * len(out_names),
                  check_rep=False),
        donate_argnums=donate, keep_unused=True,
    )
    _CACHE["fn"] = (sharded, in_names, out_names, zero_outs, mesh)
    return _CACHE["fn"]


def kernel(x, W_fc, b_fc, W_lif, b_lif):
    global LAST_EXEC_S
    if np.any(b_fc != 0) or np.any(b_lif != 0):
        return _numpy_fallback(x, W_fc, b_fc, W_lif, b_lif)
    import time
    import jax

    Ws = float(W_lif[0, 0]) + float(W_lif[0, 1]) + float(W_lif[0, 2])
    # lhsT layout: wt[j, hf, p, sh, m] = (Ws*W_fc).T[(hf*16+sh)*128+p,
    #                                               h*2048 + j*128 + m]
    Wt = np.ascontiguousarray((W_fc.astype(np.float32) * np.float32(Ws)).T)
    wt_h = []
    for h in range(CS):
        arr = Wt[:, h * MC:(h + 1) * MC].reshape(2, 16, 128, JC, 128)
        wt_h.append(np.ascontiguousarray(arr.transpose(3, 0, 2, 1, 4)))

    per_core = {"wt": [], "xt": []}
    for c in range(NCORES):
        h, q = c // BS, c % BS
        xs = np.ascontiguousarray(
            x[q * BL:(q + 1) * BL].astype(np.float32).transpose(2, 1, 0)
        ).reshape(KS, 128, N)  # [s, p, t*BL+b]
        per_core["xt"].append(np.ascontiguousarray(xs.transpose(1, 0, 2)))
        per_core["wt"].append(wt_h[h])

    sharded, in_names, out_names, zero_outs, mesh = _get_runner()
    concat_in = [np.concatenate(per_core[n], axis=0) for n in in_names]
    concat_zero = [np.concatenate([z] * NCORES, axis=0) for z in zero_outs]

    from jax.sharding import NamedSharding, PartitionSpec
    shd = NamedSharding(mesh, PartitionSpec("core"))
    args = [jax.device_put(a, shd) for a in concat_in + concat_zero]
    for a in args:
        a.block_until_ready()
    t0 = time.time()
    out_arrs = sharded(*args)
    jax.block_until_ready(out_arrs)
    LAST_EXEC_S = time.time() - t0
    out_arrs = [np.asarray(o) for o in out_arrs]

    sp_all = out_arrs[out_names.index("sp")]            # [8*128, T, JC*BL]
    out = np.empty((B, T, C), dtype=np.float32)
    for c in range(NCORES):
        h, q = c // BS, c % BS
        sp = sp_all[c * 128:(c + 1) * 128]              # [128, T, JC*BL]
        arr = sp.reshape(128, T, JC, BL)                # (p, t, j, b)
        spikes = 1.0 - np.transpose(arr, (3, 1, 2, 0))  # (b, t, j, p)
        out[q * BL:(q + 1) * BL, :, h * MC:(h + 1) * MC] = \
            spikes.reshape(BL, T, MC)
    return out
